# revision 12
# baseline (speedup 1.0000x reference)
"""Trainium2 Bass kernel for nn_ConcatLayer: (N, 9) -> (N, 3).

Pure data-parallel: the batch dim is sharded across 8 NeuronCores; each core
runs an identical elementwise Bass/Tile program over its shard.

Algorithm (bit-exact reformulation of the reference, verified vs jax):
  per row v(9,) split into segments u=v[0:3], n=v[3:6], d=v[6:9]:
    m_s  = (s0 > max(s1,s2)) - (s2 > max(s0,s1))        in {-1,0,1}
    calc = m_n^2 * (m_u + m_n + m_d); sgn = clip(calc,-1,1)
    col  = 1 if calc==0 else 0 if calc==1 else 2
    z_s  = (m_s == sgn); cmp_s = z_s * v[s][col]
    row  = first argmax(cmp_u, cmp_n, cmp_d)
    out  = v[row] * z_row

v2 layout: the host transposes each core's shard to component-plane form
(9, rows); the kernel loads 9 dense (128, F) planes per tile so every
engine op runs on long unit-stride APs (measured: strided inner-dim-3 APs
cost ~1.5x on DVE, and per-instruction overhead is ~212ns, so ops are
batched 3-planes-wide wherever segments share an operation).  Exact small
integers (masks, m, calc) are kept in bf16 for the 2x DVE TT mode; pred
masks are uint8 (BIR requirement); value planes stay fp32 for bit-exact
compares and copies.  Maxes and the two dense multiplies run on Pool
(gpsimd), base copies on the Scalar engine, so DVE - the bottleneck -
only runs compares, the stt, small bf16 algebra, and pred-copies.
"""

import numpy as np

import concourse.bass as bass
import concourse.mybir as mybir
from concourse.alu_op_type import AluOpType as A
from concourse.tile import TileContext
from concourse.bass_utils import run_bass_kernel_spmd

P = 128
N_CORES = 8
FP32 = mybir.dt.float32
BF16 = mybir.dt.bfloat16
U8 = mybir.dt.uint8


def build_kernel(rows_per_core: int, f: int, reps: int = 1,
                 wbufs: int = 1, act_sgn: bool = False) -> bass.Bass:
    """Per-core program over plane-layout input x:(9, rows) -> y:(3, rows).

    rows_per_core must equal 128*f*ntiles.  reps > 1 wraps the (idempotent)
    computation in a hardware loop for slope benchmarking.
    """
    assert rows_per_core % (P * f) == 0
    ntiles = rows_per_core // (P * f)

    nc = bass.Bass()
    x = nc.declare_dram_parameter("x", [9, rows_per_core], FP32, isOutput=False)
    y = nc.declare_dram_parameter("y", [3, rows_per_core], FP32, isOutput=True)

    from contextlib import nullcontext
    with TileContext(nc) as tc:
        with (
            tc.tile_pool(name="io", bufs=2) as io,
            tc.tile_pool(name="wk", bufs=wbufs) as wk,
            tc.For_i(0, reps, 1) if reps > 1 else nullcontext(),
        ):
            for t in range(ntiles):
                c0 = t * P * f
                c1 = (t + 1) * P * f

                # SBUF plane order j = role*3 + seg  (AA|BB|CC, each 3F).
                xt = io.tile([P, 9, f], FP32, tag="xt")
                nc.sync.dma_start(
                    out=xt[:],
                    in_=x[:, c0:c1].rearrange("j (p f) -> p j f", p=P),
                )
                AA, BB, CC = xt[:, 0:3, :], xt[:, 3:6, :], xt[:, 6:9, :]
                # per-segment component views (middle dim stride 3F)
                xt4 = xt[:].rearrange("p (r s) f -> p r s f", s=3)
                UU, NN, DD = xt4[:, :, 0, :], xt4[:, :, 1, :], xt4[:, :, 2, :]

                w1 = wk.tile([P, 3, f], FP32, tag="w1")
                w2 = wk.tile([P, 3, f], FP32, tag="w2")
                pp = wk.tile([P, 3, f], BF16, tag="pp")
                qq = wk.tile([P, 3, f], BF16, tag="qq")
                mm = wk.tile([P, 3, f], BF16, tag="mm")
                zz = wk.tile([P, 3, f], BF16, tag="zz")
                SEL = wk.tile([P, 3, f], FP32, tag="SEL")
                CMP = wk.tile([P, 3, f], FP32, tag="CMP")
                sm = wk.tile([P, 6, f], BF16, tag="sm")
                t_s, t2_s, an_s, calc_s, sgn_s, zw_s = (
                    sm[:, i:i + 1, :] for i in range(6))
                gg = wk.tile([P, 3, f], BF16, tag="gg")
                gun_s, gud_s, gnd_s = (gg[:, i:i + 1, :] for i in range(3))
                msk = wk.tile([P, 4, f], U8, tag="msk")
                e1_s, e0_s, bu_s, bn_s = (msk[:, i:i + 1, :] for i in range(4))
                ot = io.tile([P, 3, f], FP32, tag="ot")

                m_u, m_n, m_d = (mm[:, i:i + 1, :] for i in range(3))
                z_u, z_n, z_d = (zz[:, i:i + 1, :] for i in range(3))
                cmp_u, cmp_n, cmp_d = (CMP[:, i:i + 1, :] for i in range(3))

                # --- segment max-index mm = pp - qq (batched 3F) ---
                nc.vector.tensor_tensor(out=w1[:], in0=BB, in1=CC, op=A.max)
                nc.vector.tensor_tensor(out=w2[:], in0=AA, in1=BB, op=A.max)
                nc.vector.tensor_tensor(out=pp[:], in0=AA, in1=w1[:], op=A.is_gt)
                nc.vector.tensor_tensor(out=qq[:], in0=CC, in1=w2[:], op=A.is_gt)
                nc.vector.scalar_tensor_tensor(
                    out=mm[:], in0=qq[:], scalar=-1.0, in1=pp[:],
                    op0=A.mult, op1=A.add)

                # --- calc, sgn, col masks (F-sized, exact bf16 ints) ---
                nc.vector.tensor_tensor(out=t_s, in0=m_u, in1=m_d, op=A.add)
                nc.vector.tensor_tensor(out=t2_s, in0=t_s, in1=m_n, op=A.add)
                if act_sgn:
                    nc.scalar.square(out=an_s, in_=m_n)
                else:
                    nc.vector.tensor_tensor(out=an_s, in0=m_n, in1=m_n,
                                            op=A.mult)
                nc.vector.tensor_tensor(out=calc_s, in0=an_s, in1=t2_s, op=A.mult)
                if act_sgn:
                    nc.scalar.sign(out=sgn_s, in_=calc_s)
                else:
                    nc.vector.tensor_scalar(
                        out=sgn_s, in0=calc_s, scalar1=-1.0, scalar2=1.0,
                        op0=A.max, op1=A.min)
                nc.vector.tensor_scalar(
                    out=e1_s, in0=calc_s, scalar1=1.0, scalar2=None,
                    op0=A.is_equal)
                nc.vector.tensor_scalar(
                    out=e0_s, in0=calc_s, scalar1=0.0, scalar2=None,
                    op0=A.is_equal)

                # --- column select: SEL[s] = v[s][col] (batched) ---
                nc.scalar.copy(out=SEL[:], in_=CC)
                nc.vector.copy_predicated(
                    out=SEL[:], mask=e1_s.broadcast_to([P, 3, f]), data=AA)
                nc.vector.copy_predicated(
                    out=SEL[:], mask=e0_s.broadcast_to([P, 3, f]), data=BB)

                # --- z gates and gated comparands (batched) ---
                nc.vector.tensor_tensor(
                    out=zz[:], in0=mm[:], in1=sgn_s.broadcast_to([P, 3, f]),
                    op=A.is_equal)
                nc.vector.tensor_tensor(out=CMP[:], in0=zz[:], in1=SEL[:],
                                        op=A.mult)

                # --- first-argmax row masks ---
                nc.vector.tensor_tensor(out=gun_s, in0=cmp_u, in1=cmp_n, op=A.is_ge)
                nc.vector.tensor_tensor(out=gud_s, in0=cmp_u, in1=cmp_d, op=A.is_ge)
                nc.vector.tensor_tensor(out=gnd_s, in0=cmp_n, in1=cmp_d, op=A.is_ge)
                nc.vector.tensor_tensor(out=bu_s, in0=gun_s, in1=gud_s, op=A.mult)
                nc.vector.tensor_tensor(out=bn_s, in0=gnd_s, in1=bu_s, op=A.is_gt)

                # --- winner z gate ---
                nc.scalar.copy(out=zw_s, in_=z_d)
                nc.vector.copy_predicated(out=zw_s, mask=bn_s, data=z_n)
                nc.vector.copy_predicated(out=zw_s, mask=bu_s, data=z_u)

                # --- output: winner segment * zw (batched 3F) ---
                nc.scalar.copy(out=ot[:], in_=DD)
                nc.vector.copy_predicated(
                    out=ot[:], mask=bn_s.broadcast_to([P, 3, f]), data=NN)
                nc.vector.copy_predicated(
                    out=ot[:], mask=bu_s.broadcast_to([P, 3, f]), data=UU)
                nc.vector.tensor_tensor(
                    out=ot[:], in0=ot[:], in1=zw_s.broadcast_to([P, 3, f]),
                    op=A.mult)

                nc.sync.dma_start(
                    out=y[:, c0:c1].rearrange("c (p f) -> p c f", p=P),
                    in_=ot[:],
                )

    return nc


def legalize_multi_waits(nc: bass.Bass) -> None:
    """Split multi-wait sync_info into standalone EventSemaphore instructions.

    The walrus build in this environment encodes at most ONE sync-wait per
    instruction ("Too many sync wait commands" in codegen otherwise), while
    Tile emits one wait per depended-on semaphore.  Hoist all but the last
    wait onto dedicated same-engine wait instructions placed immediately
    before, which preserves per-engine program order and thus semantics.
    """
    n = 0
    for fn in nc.m.functions:
        for bb in fn.blocks:
            new_insts = []
            for inst in bb.instructions:
                si = inst.sync_info
                if si is not None and si.on_wait and len(si.on_wait) > 1:
                    waits = list(si.on_wait)
                    for w in waits[:-1]:
                        n += 1
                        new_insts.append(
                            mybir.InstEventSemaphore(
                                name=f"WSPLIT-{n}",
                                engine=inst.engine,
                                ins=[],
                                outs=[],
                                sync_info=mybir.SyncInfo(
                                    on_wait=[w], on_update=[]
                                ),
                            )
                        )
                    inst.sync_info = mybir.SyncInfo(
                        on_wait=[waits[-1]], on_update=list(si.on_update)
                    )
                new_insts.append(inst)
            bb.instructions = new_insts


_CACHED = {}

F_TILE = 1024
W_BUFS = 1


def _get_kernel(rows_per_core: int, f: int, wbufs: int = W_BUFS) -> bass.Bass:
    key = (rows_per_core, f, wbufs)
    if key not in _CACHED:
        nc = build_kernel(rows_per_core, f, wbufs=wbufs)
        nc.finalize()
        legalize_multi_waits(nc)
        _CACHED[key] = nc
    return _CACHED[key]


# SBUF plane order j = role*3 + seg maps to input column c = seg*3 + role.
_PERM = np.array([0, 3, 6, 1, 4, 7, 2, 5, 8])


def _marshal_in(x: np.ndarray):
    """Shard rows across cores; transpose each shard to role-major planes."""
    n = x.shape[0]
    rpc = n // N_CORES
    return rpc, [
        np.ascontiguousarray(x[i * rpc:(i + 1) * rpc].T[_PERM])
        for i in range(N_CORES)
    ]


def _marshal_out(results) -> np.ndarray:
    return np.concatenate(
        [np.ascontiguousarray(r["y"].T) for r in results], axis=0)


def kernel(x: np.ndarray) -> np.ndarray:
    x = np.ascontiguousarray(np.asarray(x), dtype=np.float32)
    assert x.shape[0] % N_CORES == 0
    rpc, shards = _marshal_in(x)
    nc = _get_kernel(rpc, F_TILE)
    res = run_bass_kernel_spmd(
        nc, [{"x": s} for s in shards], list(range(N_CORES))).results
    return _marshal_out(res)


def kernel_profiled(x: np.ndarray, tmpdir: str | None = None):
    """Traced run: returns (exec_time_ns, full output).  Not used by grading."""
    x = np.ascontiguousarray(np.asarray(x), dtype=np.float32)
    rpc, shards = _marshal_in(x)
    nc = _get_kernel(rpc, F_TILE)
    br = run_bass_kernel_spmd(
        nc, [{"x": s} for s in shards], list(range(N_CORES)),
        trace=True, tmpdir=tmpdir)
    if br.instructions_and_trace is not None:
        print(f"trace: {br.instructions_and_trace[1]}")
    out = _marshal_out(br.results) if br.results else None
    return br.exec_time_ns, out
